# revision 4
# baseline (speedup 1.0000x reference)
"""Trainium2 Bass kernel for cross multi-head attention (B=4, Lq=Lk=1024,
EMB=1024, 16 heads, depth 64) returning (out, att) like the reference.

Sharding: 8 cores = 4 batches x 2 head-groups (8 heads each). Each core
computes its batch's projections for its 8 heads, full attention, and
writes its slice of att [8,1024,1024] and out [1024,512].

All host-side layout prep (transposes, fp16 casts, mask scaling) is done in
numpy inside kernel(); the device kernel is pure compute.
"""

import functools
from contextlib import ExitStack

import numpy as np

import concourse.bass as bass
import concourse.mybir as mybir
import concourse.tile as tile
from concourse import bacc
from concourse.bass import ts
from concourse.bass_utils import run_bass_kernel_spmd
from concourse.masks import make_identity

B, LQ, LK = 4, 1024, 1024
QDIM = KVDIM = 512
EMB, HEADS, DEPTH = 1024, 16, 64
SCALE = EMB ** (-0.5)  # 1/32
NCORES = 8
HPC = 8          # heads per core
GEMB = 512       # emb cols per core
MASKVAL = -1920.0  # pre-scale mask bias; exp sees -1920/32 = -60

F16 = mybir.dt.float16
F32 = mybir.dt.float32


def _body(tc, ins, outs):
    nc = tc.nc
    EXP = mybir.ActivationFunctionType.Exp

    with ExitStack() as ctx:
        singles = ctx.enter_context(tc.tile_pool(name="singles", bufs=1))

        ident = singles.tile([128, 128], F16, name="ident")
        make_identity(nc, ident)
        onescol = singles.tile([1, 128], F16, name="onescol")
        nc.vector.memset(onescol, 1.0)

        # inputs -> SBUF
        xq = singles.tile([128, 4, 1024], F16, name="xq")      # [c%128, cchunk, i]
        nc.sync.dma_start(out=xq, in_=ins["xqt"].rearrange("(cc p) i -> p cc i", p=128))
        xkv = singles.tile([128, 4, 1024], F16, name="xkv")
        nc.sync.dma_start(out=xkv, in_=ins["xkvt"].rearrange("(cc p) i -> p cc i", p=128))
        wq = singles.tile([128, 4, 512], F16, name="wq")       # [c%128, cchunk, dout]
        nc.sync.dma_start(out=wq, in_=ins["wqt"].rearrange("(cc p) d -> p cc d", p=128))
        wk = singles.tile([128, 4, 512], F16, name="wk")
        nc.sync.dma_start(out=wk, in_=ins["wkt"].rearrange("(cc p) d -> p cc d", p=128))
        wv = singles.tile([128, 4, 512], F16, name="wv")
        nc.sync.dma_start(out=wv, in_=ins["wvt"].rearrange("(cc p) d -> p cc d", p=128))
        qb = singles.tile([128, 4], F32, name="qb")            # [dout%128, dchunk]
        nc.sync.dma_start(out=qb, in_=ins["qb"].rearrange("(dc p) -> p dc", p=128))
        kb = singles.tile([128, 4], F32, name="kb")
        nc.sync.dma_start(out=kb, in_=ins["kb"].rearrange("(dc p) -> p dc", p=128))
        vb = singles.tile([1, 512], F16, name="vb")
        nc.sync.dma_start(out=vb, in_=ins["vb"][None, :])
        maskb = singles.tile([128, 8, 1024], F16, name="maskb")  # [i%128, ichunk, j]
        nc.sync.dma_start(out=maskb, in_=ins["maskb"].rearrange("(ic p) j -> p ic j", p=128))

        # persistent intermediates
        qt = singles.tile([128, 4, 1024], F16, name="qt")   # QT: [d%128, dchunk, i]
        kt = singles.tile([128, 4, 1024], F16, name="kt")   # KT: [d%128, dchunk, j]
        v = singles.tile([128, 8, 512], F16, name="v")      # V:  [j%128, jchunk, dv]
        out_sb = singles.tile([128, 8, 512], F16, name="out_sb")  # [i%128, ichunk, emb]

        # ---- projections ----
        with tc.tile_pool(name="pp", bufs=2, space="PSUM") as pp:
            for dc in range(4):
                qp = pp.tile([128, 1024], F32, tag="qkp", name="qp")
                for cc in range(4):
                    for nh in range(2):
                        nc.tensor.matmul(
                            qp[:, ts(nh, 512)],
                            lhsT=wq[:, cc, ts(dc, 128)],
                            rhs=xq[:, cc, ts(nh, 512)],
                            start=(cc == 0), stop=(cc == 3),
                        )
                nc.vector.tensor_scalar_add(qt[:, dc, :], qp, qb[:, dc:dc + 1])
                kp = pp.tile([128, 1024], F32, tag="qkp", name="kp")
                for cc in range(4):
                    for nh in range(2):
                        nc.tensor.matmul(
                            kp[:, ts(nh, 512)],
                            lhsT=wk[:, cc, ts(dc, 128)],
                            rhs=xkv[:, cc, ts(nh, 512)],
                            start=(cc == 0), stop=(cc == 3),
                        )
                nc.vector.tensor_scalar_add(kt[:, dc, :], kp, kb[:, dc:dc + 1])
            for jc in range(8):
                vp = pp.tile([128, 512], F32, tag="vp", name="vp")
                # bias via rank-1 ones x vb
                nc.tensor.matmul(vp, lhsT=onescol, rhs=vb, start=True, stop=False)
                for cc in range(4):
                    nc.tensor.matmul(
                        vp,
                        lhsT=xkv[:, cc, ts(jc, 128)],
                        rhs=wv[:, cc, :],
                        start=False, stop=(cc == 3),
                    )
                nc.vector.tensor_copy(v[:, jc, :], vp)

        # ---- attention ----
        att_dram = outs["att_part"]
        with (
            tc.tile_pool(name="sp", bufs=2, space="PSUM") as sp,
            tc.tile_pool(name="op", bufs=2, space="PSUM") as opool,
            tc.tile_pool(name="attp", bufs=2) as attp,
            tc.tile_pool(name="wkp", bufs=3) as wkp,
        ):
            for h in range(HPC):
                dc, po = h // 2, (h % 2) * 64
                att_h = attp.tile([128, 8, 1024], F16, tag="att_h", name="att_h")
                attT_h = attp.tile([128, 8, 1024], F16, tag="attT_h", name="attT_h")
                for ic in range(8):
                    s = sp.tile([128, 1024], F32, tag="s", name="s")
                    for nh in range(2):
                        nc.tensor.matmul(
                            s[:, ts(nh, 512)],
                            lhsT=ident,
                            rhs=maskb[:, ic, ts(nh, 512)],
                            start=True, stop=False,
                        )
                        nc.tensor.matmul(
                            s[:, ts(nh, 512)],
                            lhsT=qt[po:po + 64, dc, ts(ic, 128)],
                            rhs=kt[po:po + 64, dc, ts(nh, 512)],
                            start=False, stop=True,
                        )
                    p = wkp.tile([128, 1024], F16, tag="p", name="p")
                    z = wkp.tile([128, 1], F32, tag="z", name="z")
                    nc.scalar.activation(out=p, in_=s, func=EXP, scale=SCALE, accum_out=z)
                    r = wkp.tile([128, 1], F32, tag="r", name="r")
                    nc.vector.reciprocal(r, z)
                    nc.vector.tensor_scalar_mul(att_h[:, ic, :], p, r)
                    for jc in range(8):
                        nc.sync.dma_start(
                            out=attT_h[:, jc, ts(ic, 128)],
                            in_=att_h[:, ic, ts(jc, 128)],
                            transpose=True,
                        )
                # att out (fp16 -> fp32 cast during DMA)
                nc.gpsimd.dma_start(
                    out=att_dram[h].rearrange("(ic p) j -> p ic j", p=128),
                    in_=att_h,
                )
                # O^T = V_h^T @ attT
                ot = opool.tile([64, 1024], F32, tag="ot", name="ot")
                for jc in range(8):
                    for nh in range(2):
                        nc.tensor.matmul(
                            ot[:, ts(nh, 512)],
                            lhsT=v[:, jc, ts(h, 64)],
                            rhs=attT_h[:, jc, ts(nh, 512)],
                            start=(jc == 0), stop=(jc == 7),
                        )
                otf = wkp.tile([64, 1024], F16, tag="otf", name="otf")
                nc.vector.tensor_copy(otf, ot)
                for ic in range(8):
                    nc.sync.dma_start(
                        out=out_sb[:, ic, ts(h, 64)],
                        in_=otf[:, ts(ic, 128)],
                        transpose=True,
                    )
            nc.gpsimd.dma_start(
                out=outs["out_part"].rearrange("(ic p) d -> p ic d", p=128),
                in_=out_sb,
            )


@functools.cache
def _build():
    nc = bacc.Bacc(
        "TRN2",
        target_bir_lowering=False,
        debug=False,
        enable_asserts=False,
        num_devices=NCORES,
    )
    ins = {
        "xqt": nc.dram_tensor("xqt", [QDIM, LQ], F16, kind="ExternalInput").ap(),
        "xkvt": nc.dram_tensor("xkvt", [KVDIM, LK], F16, kind="ExternalInput").ap(),
        "wqt": nc.dram_tensor("wqt", [QDIM, GEMB], F16, kind="ExternalInput").ap(),
        "wkt": nc.dram_tensor("wkt", [KVDIM, GEMB], F16, kind="ExternalInput").ap(),
        "wvt": nc.dram_tensor("wvt", [KVDIM, GEMB], F16, kind="ExternalInput").ap(),
        "qb": nc.dram_tensor("qb", [GEMB], F32, kind="ExternalInput").ap(),
        "kb": nc.dram_tensor("kb", [GEMB], F32, kind="ExternalInput").ap(),
        "vb": nc.dram_tensor("vb", [GEMB], F16, kind="ExternalInput").ap(),
        "maskb": nc.dram_tensor("maskb", [LQ, LK], F16, kind="ExternalInput").ap(),
    }
    outs = {
        "att_part": nc.dram_tensor("att_part", [HPC, LQ, LK], F32, kind="ExternalOutput").ap(),
        "out_part": nc.dram_tensor("out_part", [LQ, GEMB], F32, kind="ExternalOutput").ap(),
    }
    with tile.TileContext(nc) as tc:
        _body(tc, ins, outs)
    nc.compile()
    return nc


def _prep_in_maps(q_candidate, kv_candidate, pad_mask, Wq_w, Wq_b, Wk_w, Wk_b, Wv_w, Wv_b):
    q_candidate = np.asarray(q_candidate, dtype=np.float32)
    kv_candidate = np.asarray(kv_candidate, dtype=np.float32)
    pad_mask = np.asarray(pad_mask)
    Wq_w = np.asarray(Wq_w, dtype=np.float32)
    Wk_w = np.asarray(Wk_w, dtype=np.float32)
    Wv_w = np.asarray(Wv_w, dtype=np.float32)
    Wq_b = np.asarray(Wq_b, dtype=np.float32)
    Wk_b = np.asarray(Wk_b, dtype=np.float32)
    Wv_b = np.asarray(Wv_b, dtype=np.float32)

    in_maps = []
    for c in range(NCORES):
        b, g = divmod(c, 2)
        sl = slice(g * GEMB, (g + 1) * GEMB)
        in_maps.append({
            "xqt": np.ascontiguousarray(q_candidate[b].T.astype(np.float16)),
            "xkvt": np.ascontiguousarray(kv_candidate[b].T.astype(np.float16)),
            "wqt": np.ascontiguousarray(Wq_w[sl].T.astype(np.float16)),
            "wkt": np.ascontiguousarray(Wk_w[sl].T.astype(np.float16)),
            "wvt": np.ascontiguousarray(Wv_w[sl].T.astype(np.float16)),
            "qb": np.ascontiguousarray(Wq_b[sl]),
            "kb": np.ascontiguousarray(Wk_b[sl]),
            "vb": np.ascontiguousarray(Wv_b[sl].astype(np.float16)),
            "maskb": np.ascontiguousarray(pad_mask[b].astype(np.float16) * np.float16(MASKVAL)),
        })
    return in_maps


def run(trace=False, **inputs):
    in_maps = _prep_in_maps(**inputs)
    nc = _build()
    res = run_bass_kernel_spmd(nc, in_maps, core_ids=list(range(NCORES)), trace=trace)
    out = np.empty((B, LQ, EMB), np.float32)
    att = np.empty((B, HEADS, LQ, LK), np.float32)
    for c in range(NCORES):
        b, g = divmod(c, 2)
        att[b, g * HPC:(g + 1) * HPC] = res.results[c]["att_part"]
        out[b, :, g * GEMB:(g + 1) * GEMB] = res.results[c]["out_part"]
    return (out, att), res


def kernel(**inputs):
    (out, att), _ = run(trace=False, **inputs)
    return out, att


# revision 5
# speedup vs baseline: 2.6808x; 2.6808x over previous
"""Trainium2 Bass kernel for cross multi-head attention (B=4, Lq=Lk=1024,
EMB=1024, 16 heads, depth 64) returning (out, att) like the reference.

Sharding: 8 cores = 4 batches x 2 head-groups (8 heads each). Each core
computes its batch's projections for its 8 heads, full attention, and
writes its slice of att [8,1024,1024] and out [1024,512].

All host-side layout prep (transposes, 16-bit casts, mask scaling) is done
in numpy inside kernel(); the device kernel is pure compute.

Device dataflow per core:
  - QT/KT (head-dim-major) and V (seq-major) projections on PE, bf16.
  - Per (head, 128-query chunk): S = mask*(-1920) + Q.K^T accumulated in
    PSUM (mask added via identity matmul), exp((S)/32) on ACT with fused
    row-sum (softmax denominator), reciprocal + row-scale on DVE.
  - att tiles written to DRAM as fp32 via SWDGE cast-DMA; also
    xbar-transposed (one 3D dma_start_transpose per tile) to feed
    out = att @ V on PE; out^T xbar-transposed back and cast-written.
"""

import functools
from contextlib import ExitStack

import numpy as np

import concourse.bass as bass
import concourse.mybir as mybir
import concourse.tile as tile
from concourse import bacc
from concourse.bass import ts
from concourse.bass_utils import run_bass_kernel_spmd
from concourse.masks import make_identity

B, LQ, LK = 4, 1024, 1024
QDIM = KVDIM = 512
EMB, HEADS, DEPTH = 1024, 16, 64
SCALE = EMB ** (-0.5)  # 1/32
NCORES = 8
HPC = 8          # heads per core
GEMB = 512       # emb cols per core
MASKVAL = -1920.0  # pre-scale mask bias; exp sees -1920/32 = -60

F16 = mybir.dt.bfloat16
NP16 = np.dtype("bfloat16") if hasattr(np, "bfloat16") else None
F32 = mybir.dt.float32


def _np16():
    import ml_dtypes

    return np.dtype(ml_dtypes.bfloat16)


def _body(tc, ins, outs):
    nc = tc.nc
    EXP = mybir.ActivationFunctionType.Exp
    rings = [nc.sync, nc.scalar]  # two HWDGE rings for xbar transposes

    with ExitStack() as ctx:
        singles = ctx.enter_context(tc.tile_pool(name="singles", bufs=1))

        ident = singles.tile([128, 128], F16, name="ident")
        make_identity(nc, ident)
        onescol = singles.tile([1, 128], F16, name="onescol")
        nc.vector.memset(onescol, 1.0)

        # inputs -> SBUF
        xq = singles.tile([128, 4, 1024], F16, name="xq")      # [c%128, cchunk, i]
        nc.sync.dma_start(out=xq, in_=ins["xqt"].rearrange("(cc p) i -> p cc i", p=128))
        xkv = singles.tile([128, 4, 1024], F16, name="xkv")
        nc.sync.dma_start(out=xkv, in_=ins["xkvt"].rearrange("(cc p) i -> p cc i", p=128))
        wq = singles.tile([128, 4, 512], F16, name="wq")       # [c%128, cchunk, dout]
        nc.scalar.dma_start(out=wq, in_=ins["wqt"].rearrange("(cc p) d -> p cc d", p=128))
        wk = singles.tile([128, 4, 512], F16, name="wk")
        nc.scalar.dma_start(out=wk, in_=ins["wkt"].rearrange("(cc p) d -> p cc d", p=128))
        wv = singles.tile([128, 4, 512], F16, name="wv")
        nc.scalar.dma_start(out=wv, in_=ins["wvt"].rearrange("(cc p) d -> p cc d", p=128))
        qb = singles.tile([128, 4], F32, name="qb")            # [dout%128, dchunk]
        nc.sync.dma_start(out=qb, in_=ins["qb"].rearrange("(dc p) -> p dc", p=128))
        kb = singles.tile([128, 4], F32, name="kb")
        nc.sync.dma_start(out=kb, in_=ins["kb"].rearrange("(dc p) -> p dc", p=128))
        vb = singles.tile([1, 512], F16, name="vb")
        nc.sync.dma_start(out=vb, in_=ins["vb"][None, :])
        maskb = singles.tile([128, 8, 1024], F16, name="maskb")  # [i%128, ichunk, j]
        nc.sync.dma_start(out=maskb, in_=ins["maskb"].rearrange("(ic p) j -> p ic j", p=128))
        # warm DVE on the bias tiles so later tensor_scalar ops carry 1 wait
        qbw = singles.tile([128, 4], F32, name="qbw")
        nc.vector.tensor_copy(qbw, qb)
        kbw = singles.tile([128, 4], F32, name="kbw")
        nc.vector.tensor_copy(kbw, kb)

        # persistent intermediates
        qt = singles.tile([128, 4, 1024], F16, name="qt")   # QT: [d%128, dchunk, i]
        kt = singles.tile([128, 4, 1024], F16, name="kt")   # KT: [d%128, dchunk, j]
        v = singles.tile([128, 8, 512], F16, name="v")      # V:  [j%128, jchunk, dv]
        out_sb = singles.tile([128, 8, 512], F16, name="out_sb")  # [i%128, ichunk, emb]

        # ---- projections ----
        with tc.tile_pool(name="pp", bufs=2, space="PSUM") as pp:
            for dc in range(4):
                qp = pp.tile([128, 1024], F32, tag="qkp", name="qp")
                for cc in range(4):
                    for nh in range(2):
                        nc.tensor.matmul(
                            qp[:, ts(nh, 512)],
                            lhsT=wq[:, cc, ts(dc, 128)],
                            rhs=xq[:, cc, ts(nh, 512)],
                            start=(cc == 0), stop=(cc == 3),
                        )
                nc.vector.tensor_scalar_add(qt[:, dc, :], qp, qbw[:, dc:dc + 1])
                kp = pp.tile([128, 1024], F32, tag="qkp", name="kp")
                for cc in range(4):
                    for nh in range(2):
                        nc.tensor.matmul(
                            kp[:, ts(nh, 512)],
                            lhsT=wk[:, cc, ts(dc, 128)],
                            rhs=xkv[:, cc, ts(nh, 512)],
                            start=(cc == 0), stop=(cc == 3),
                        )
                nc.vector.tensor_scalar_add(kt[:, dc, :], kp, kbw[:, dc:dc + 1])
            for jc in range(8):
                vp = pp.tile([128, 512], F32, tag="vp", name="vp")
                # bias via rank-1 ones x vb
                nc.tensor.matmul(vp, lhsT=onescol, rhs=vb, start=True, stop=False)
                for cc in range(4):
                    nc.tensor.matmul(
                        vp,
                        lhsT=xkv[:, cc, ts(jc, 128)],
                        rhs=wv[:, cc, :],
                        start=False, stop=(cc == 3),
                    )
                nc.vector.tensor_copy(v[:, jc, :], vp)

        # ---- attention ----
        att_dram = outs["att_part"]
        with (
            tc.tile_pool(name="sp", bufs=2, space="PSUM") as sp,
            tc.tile_pool(name="op", bufs=2, space="PSUM") as opool,
            tc.tile_pool(name="attp", bufs=2) as attp,
            tc.tile_pool(name="wkp", bufs=3) as wkp,
        ):
            for h in range(HPC):
                dc, po = h // 2, (h % 2) * 64
                att_h = attp.tile([128, 8, 1024], F16, tag="att_h", name="att_h")
                attT_h = attp.tile([128, 8, 1024], F16, tag="attT_h", name="attT_h")
                for ic in range(8):
                    s = sp.tile([128, 1024], F32, tag="s", name="s")
                    for nh in range(2):
                        nc.tensor.matmul(
                            s[:, ts(nh, 512)],
                            lhsT=ident,
                            rhs=maskb[:, ic, ts(nh, 512)],
                            start=True, stop=False,
                        )
                        nc.tensor.matmul(
                            s[:, ts(nh, 512)],
                            lhsT=qt[po:po + 64, dc, ts(ic, 128)],
                            rhs=kt[po:po + 64, dc, ts(nh, 512)],
                            start=False, stop=True,
                        )
                    p = wkp.tile([128, 1024], F16, tag="p", name="p")
                    z = wkp.tile([128, 1], F32, tag="z", name="z")
                    nc.scalar.activation(out=p, in_=s, func=EXP, scale=SCALE, accum_out=z)
                    r = wkp.tile([128, 1], F32, tag="r", name="r")
                    nc.vector.reciprocal(r, z)
                    nc.vector.tensor_scalar_mul(att_h[:, ic, :], p, r)
                    # one 3D xbar transpose: attT[:, jc, ic-block] = att[ic][:, jc-block].T
                    rings[(h * 8 + ic) % 2].dma_start(
                        out=attT_h[:, :, ts(ic, 128)],
                        in_=att_h[:, ic, :],
                        transpose=True,
                    )
                # att out (16-bit -> fp32 cast during DMA)
                nc.gpsimd.dma_start(
                    out=att_dram[h].rearrange("(ic p) j -> p ic j", p=128),
                    in_=att_h,
                )
                # O^T = V_h^T @ attT
                ot = opool.tile([64, 1024], F32, tag="ot", name="ot")
                for jc in range(8):
                    for nh in range(2):
                        nc.tensor.matmul(
                            ot[:, ts(nh, 512)],
                            lhsT=v[:, jc, ts(h, 64)],
                            rhs=attT_h[:, jc, ts(nh, 512)],
                            start=(jc == 0), stop=(jc == 7),
                        )
                otf = wkp.tile([64, 1024], F16, tag="otf", name="otf")
                nc.vector.tensor_copy(otf, ot)
                rings[h % 2].dma_start(
                    out=out_sb[:, :, ts(h, 64)],
                    in_=otf,
                    transpose=True,
                )
            nc.gpsimd.dma_start(
                out=outs["out_part"].rearrange("(ic p) d -> p ic d", p=128),
                in_=out_sb,
            )


@functools.cache
def _build():
    nc = bacc.Bacc(
        "TRN2",
        target_bir_lowering=False,
        debug=False,
        enable_asserts=False,
        num_devices=NCORES,
    )
    ins = {
        "xqt": nc.dram_tensor("xqt", [QDIM, LQ], F16, kind="ExternalInput").ap(),
        "xkvt": nc.dram_tensor("xkvt", [KVDIM, LK], F16, kind="ExternalInput").ap(),
        "wqt": nc.dram_tensor("wqt", [QDIM, GEMB], F16, kind="ExternalInput").ap(),
        "wkt": nc.dram_tensor("wkt", [KVDIM, GEMB], F16, kind="ExternalInput").ap(),
        "wvt": nc.dram_tensor("wvt", [KVDIM, GEMB], F16, kind="ExternalInput").ap(),
        "qb": nc.dram_tensor("qb", [GEMB], F32, kind="ExternalInput").ap(),
        "kb": nc.dram_tensor("kb", [GEMB], F32, kind="ExternalInput").ap(),
        "vb": nc.dram_tensor("vb", [GEMB], F16, kind="ExternalInput").ap(),
        "maskb": nc.dram_tensor("maskb", [LQ, LK], F16, kind="ExternalInput").ap(),
    }
    outs = {
        "att_part": nc.dram_tensor("att_part", [HPC, LQ, LK], F32, kind="ExternalOutput").ap(),
        "out_part": nc.dram_tensor("out_part", [LQ, GEMB], F32, kind="ExternalOutput").ap(),
    }
    with tile.TileContext(nc) as tc:
        _body(tc, ins, outs)
    nc.compile()
    return nc


def _prep_in_maps(q_candidate, kv_candidate, pad_mask, Wq_w, Wq_b, Wk_w, Wk_b, Wv_w, Wv_b):
    np16 = _np16()
    q_candidate = np.asarray(q_candidate, dtype=np.float32)
    kv_candidate = np.asarray(kv_candidate, dtype=np.float32)
    pad_mask = np.asarray(pad_mask)
    Wq_w = np.asarray(Wq_w, dtype=np.float32)
    Wk_w = np.asarray(Wk_w, dtype=np.float32)
    Wv_w = np.asarray(Wv_w, dtype=np.float32)
    Wq_b = np.asarray(Wq_b, dtype=np.float32)
    Wk_b = np.asarray(Wk_b, dtype=np.float32)
    Wv_b = np.asarray(Wv_b, dtype=np.float32)

    in_maps = []
    for c in range(NCORES):
        b, g = divmod(c, 2)
        sl = slice(g * GEMB, (g + 1) * GEMB)
        in_maps.append({
            "xqt": np.ascontiguousarray(q_candidate[b].T.astype(np16)),
            "xkvt": np.ascontiguousarray(kv_candidate[b].T.astype(np16)),
            "wqt": np.ascontiguousarray(Wq_w[sl].T.astype(np16)),
            "wkt": np.ascontiguousarray(Wk_w[sl].T.astype(np16)),
            "wvt": np.ascontiguousarray(Wv_w[sl].T.astype(np16)),
            "qb": np.ascontiguousarray(Wq_b[sl]),
            "kb": np.ascontiguousarray(Wk_b[sl]),
            "vb": np.ascontiguousarray(Wv_b[sl].astype(np16)),
            "maskb": np.ascontiguousarray((pad_mask[b].astype(np.float32) * MASKVAL).astype(np16)),
        })
    return in_maps


def run(trace=False, **inputs):
    in_maps = _prep_in_maps(**inputs)
    nc = _build()
    res = run_bass_kernel_spmd(nc, in_maps, core_ids=list(range(NCORES)), trace=trace)
    out = np.empty((B, LQ, EMB), np.float32)
    att = np.empty((B, HEADS, LQ, LK), np.float32)
    for c in range(NCORES):
        b, g = divmod(c, 2)
        att[b, g * HPC:(g + 1) * HPC] = res.results[c]["att_part"]
        out[b, :, g * GEMB:(g + 1) * GEMB] = res.results[c]["out_part"]
    return (out, att), res


def kernel(**inputs):
    (out, att), _ = run(trace=False, **inputs)
    return out, att


# revision 16
# speedup vs baseline: 3.0986x; 1.1558x over previous
"""Trainium2 Bass kernel for cross multi-head attention (B=4, Lq=Lk=1024,
EMB=1024, 16 heads, depth 64) returning (out, att) like the reference.

Sharding: 8 cores = 4 batches x 2 head-groups (8 heads each). Each core
computes its batch's projections for its 8 heads, full attention, and
writes its slice of att [8,1024,1024] and out [1024,512].

All host-side layout prep (transposes, 16-bit casts, mask scaling) is done
in numpy inside kernel(); the device kernel is pure compute.

Device dataflow per core:
  - QT/KT (head-dim-major) and V (seq-major) projections on PE, bf16.
  - Per (head, 128-query chunk): S = mask*(-1920) + Q.K^T accumulated in
    PSUM (mask added via identity matmul), exp((S)/32) on ACT with fused
    row-sum (softmax denominator), reciprocal + row-scale on DVE.
  - att tiles written to DRAM as fp32 via SWDGE cast-DMA; also
    xbar-transposed (one 3D dma_start_transpose per tile) to feed
    out = att @ V on PE; out^T xbar-transposed back and cast-written.
"""

import functools
from contextlib import ExitStack

import numpy as np

import concourse.bass as bass
import concourse.mybir as mybir
import concourse.tile as tile
from concourse import bacc
from concourse.bass import ts
from concourse.bass_utils import run_bass_kernel_spmd
from concourse.masks import make_identity

B, LQ, LK = 4, 1024, 1024
QDIM = KVDIM = 512
EMB, HEADS, DEPTH = 1024, 16, 64
SCALE = EMB ** (-0.5)  # 1/32
NCORES = 8
HPC = 8          # heads per core
GEMB = 512       # emb cols per core
MASKVAL = -1920.0  # pre-scale mask bias; exp sees -1920/32 = -60

F16 = mybir.dt.bfloat16
NP16 = np.dtype("bfloat16") if hasattr(np, "bfloat16") else None
F32 = mybir.dt.float32


def _np16():
    import ml_dtypes

    return np.dtype(ml_dtypes.bfloat16)


def _body(tc, ins, outs):
    nc = tc.nc
    EXP = mybir.ActivationFunctionType.Exp

    with ExitStack() as ctx:
        singles = ctx.enter_context(tc.tile_pool(name="singles", bufs=1))

        ident = singles.tile([128, 128], F16, name="ident")
        make_identity(nc, ident)
        onescol = singles.tile([1, 128], F16, name="onescol")
        nc.vector.memset(onescol, 1.0)

        # inputs -> SBUF
        xq = singles.tile([128, 4, 1024], F16, name="xq")      # [c%128, cchunk, i]
        nc.sync.dma_start(out=xq, in_=ins["xqt"].rearrange("(cc p) i -> p cc i", p=128))
        xkv = singles.tile([128, 4, 1024], F16, name="xkv")
        nc.sync.dma_start(out=xkv, in_=ins["xkvt"].rearrange("(cc p) i -> p cc i", p=128))
        wq = singles.tile([128, 4, 512], F16, name="wq")       # [c%128, cchunk, dout]
        nc.sync.dma_start(out=wq, in_=ins["wqt"].rearrange("(cc p) d -> p cc d", p=128))
        wk = singles.tile([128, 4, 512], F16, name="wk")
        nc.sync.dma_start(out=wk, in_=ins["wkt"].rearrange("(cc p) d -> p cc d", p=128))
        wv = singles.tile([128, 4, 512], F16, name="wv")
        nc.sync.dma_start(out=wv, in_=ins["wvt"].rearrange("(cc p) d -> p cc d", p=128))
        qb = singles.tile([128, 4], F32, name="qb")            # [dout%128, dchunk]
        nc.sync.dma_start(out=qb, in_=ins["qb"].rearrange("(dc p) -> p dc", p=128))
        kb = singles.tile([128, 4], F32, name="kb")
        nc.sync.dma_start(out=kb, in_=ins["kb"].rearrange("(dc p) -> p dc", p=128))
        vb = singles.tile([1, 512], F16, name="vb")
        nc.sync.dma_start(out=vb, in_=ins["vb"][None, :])
        maskb = singles.tile([128, 8, 1024], F16, name="maskb")  # [i%128, ichunk, j]
        nc.sync.dma_start(out=maskb, in_=ins["maskb"].rearrange("(ic p) j -> p ic j", p=128))
        # warm DVE on the bias tiles so later tensor_scalar ops carry 1 wait
        qbw = singles.tile([128, 4], F32, name="qbw")
        nc.vector.tensor_copy(qbw, qb)
        kbw = singles.tile([128, 4], F32, name="kbw")
        nc.vector.tensor_copy(kbw, kb)

        # persistent intermediates
        qt = singles.tile([128, 4, 1024], F16, name="qt")   # QT: [d%128, dchunk, i]
        kt = singles.tile([128, 4, 1024], F16, name="kt")   # KT: [d%128, dchunk, j]
        v = singles.tile([128, 8, 512], F16, name="v")      # V:  [j%128, jchunk, dv]
        out_sb = singles.tile([128, 8, 512], F16, name="out_sb")  # [i%128, ichunk, emb]

        # ---- projections ----
        with tc.tile_pool(name="pp", bufs=2, space="PSUM") as pp:
            for dc in range(4):
                qp = pp.tile([128, 1024], F32, tag="qkp", name="qp")
                for cc in range(4):
                    for nh in range(2):
                        nc.tensor.matmul(
                            qp[:, ts(nh, 512)],
                            lhsT=wq[:, cc, ts(dc, 128)],
                            rhs=xq[:, cc, ts(nh, 512)],
                            start=(cc == 0), stop=(cc == 3),
                        )
                nc.vector.tensor_scalar_add(qt[:, dc, :], qp, qbw[:, dc:dc + 1])
                kp = pp.tile([128, 1024], F32, tag="qkp", name="kp")
                for cc in range(4):
                    for nh in range(2):
                        nc.tensor.matmul(
                            kp[:, ts(nh, 512)],
                            lhsT=wk[:, cc, ts(dc, 128)],
                            rhs=xkv[:, cc, ts(nh, 512)],
                            start=(cc == 0), stop=(cc == 3),
                        )
                nc.vector.tensor_scalar_add(kt[:, dc, :], kp, kbw[:, dc:dc + 1])
            for jc in range(8):
                vp = pp.tile([128, 512], F32, tag="vp", name="vp")
                # bias via rank-1 ones x vb
                nc.tensor.matmul(vp, lhsT=onescol, rhs=vb, start=True, stop=False)
                for cc in range(4):
                    nc.tensor.matmul(
                        vp,
                        lhsT=xkv[:, cc, ts(jc, 128)],
                        rhs=wv[:, cc, :],
                        start=False, stop=(cc == 3),
                    )
                nc.vector.tensor_copy(v[:, jc, :], vp)

        # ---- attention ----
        att_dram = outs["att_part"]
        with (
            tc.tile_pool(name="sp", bufs=2, space="PSUM") as sp,
            tc.tile_pool(name="op", bufs=2, space="PSUM") as opool,
            tc.tile_pool(name="attp", bufs=2) as attp,
            tc.tile_pool(name="wkp", bufs=3) as wkp,
        ):
            for h in range(HPC):
                dc, po = h // 2, (h % 2) * 64
                att_h = attp.tile([128, 8, 1024], F16, tag="att_h", name="att_h")
                attT_h = attp.tile([128, 8, 1024], F16, tag="attT_h", name="attT_h")
                for ic in range(8):
                    s = sp.tile([128, 1024], F32, tag="s", name="s")
                    for nh in range(2):
                        nc.tensor.matmul(
                            s[:, ts(nh, 512)],
                            lhsT=ident,
                            rhs=maskb[:, ic, ts(nh, 512)],
                            start=True, stop=False,
                        )
                        nc.tensor.matmul(
                            s[:, ts(nh, 512)],
                            lhsT=qt[po:po + 64, dc, ts(ic, 128)],
                            rhs=kt[po:po + 64, dc, ts(nh, 512)],
                            start=False, stop=True,
                        )
                    p = wkp.tile([128, 1024], F16, tag="p", name="p")
                    z = wkp.tile([128, 1], F32, tag="z", name="z")
                    nc.scalar.activation(out=p, in_=s, func=EXP, scale=SCALE, accum_out=z)
                    r = wkp.tile([128, 1], F32, tag="r", name="r")
                    nc.vector.reciprocal(r, z)
                    nc.vector.tensor_scalar_mul(att_h[:, ic, :], p, r)
                    # one 3D xbar transpose: attT[:, jc, ic-block] = att[ic][:, jc-block].T
                    nc.sync.dma_start(
                        out=attT_h[:, :, ts(ic, 128)],
                        in_=att_h[:, ic, :],
                        transpose=True,
                    )
                # att out (16-bit -> fp32 cast during DMA)
                nc.gpsimd.dma_start(
                    out=att_dram[h].rearrange("(ic p) j -> p ic j", p=128),
                    in_=att_h,
                )
                # O^T = V_h^T @ attT
                ot = opool.tile([64, 1024], F32, tag="ot", name="ot")
                for jc in range(8):
                    for nh in range(2):
                        nc.tensor.matmul(
                            ot[:, ts(nh, 512)],
                            lhsT=v[:, jc, ts(h, 64)],
                            rhs=attT_h[:, jc, ts(nh, 512)],
                            start=(jc == 0), stop=(jc == 7),
                        )
                otf = wkp.tile([64, 1024], F16, tag="otf", name="otf")
                nc.vector.tensor_copy(otf, ot)
                nc.sync.dma_start(
                    out=out_sb[:, :, ts(h, 64)],
                    in_=otf,
                    transpose=True,
                )
            nc.gpsimd.dma_start(
                out=outs["out_part"].rearrange("(ic p) d -> p ic d", p=128),
                in_=out_sb,
            )


@functools.cache
def _build():
    nc = bacc.Bacc(
        "TRN2",
        target_bir_lowering=False,
        debug=False,
        enable_asserts=False,
        num_devices=NCORES,
    )
    ins = {
        "xqt": nc.dram_tensor("xqt", [QDIM, LQ], F16, kind="ExternalInput").ap(),
        "xkvt": nc.dram_tensor("xkvt", [KVDIM, LK], F16, kind="ExternalInput").ap(),
        "wqt": nc.dram_tensor("wqt", [QDIM, GEMB], F16, kind="ExternalInput").ap(),
        "wkt": nc.dram_tensor("wkt", [KVDIM, GEMB], F16, kind="ExternalInput").ap(),
        "wvt": nc.dram_tensor("wvt", [KVDIM, GEMB], F16, kind="ExternalInput").ap(),
        "qb": nc.dram_tensor("qb", [GEMB], F32, kind="ExternalInput").ap(),
        "kb": nc.dram_tensor("kb", [GEMB], F32, kind="ExternalInput").ap(),
        "vb": nc.dram_tensor("vb", [GEMB], F16, kind="ExternalInput").ap(),
        "maskb": nc.dram_tensor("maskb", [LQ, LK], F16, kind="ExternalInput").ap(),
    }
    outs = {
        "att_part": nc.dram_tensor("att_part", [HPC, LQ, LK], F32, kind="ExternalOutput").ap(),
        "out_part": nc.dram_tensor("out_part", [LQ, GEMB], F32, kind="ExternalOutput").ap(),
    }
    with tile.TileContext(nc) as tc:
        _body(tc, ins, outs)
    nc.compile()
    return nc


def _prep_in_maps(q_candidate, kv_candidate, pad_mask, Wq_w, Wq_b, Wk_w, Wk_b, Wv_w, Wv_b):
    np16 = _np16()
    q_candidate = np.asarray(q_candidate, dtype=np.float32)
    kv_candidate = np.asarray(kv_candidate, dtype=np.float32)
    pad_mask = np.asarray(pad_mask)
    Wq_w = np.asarray(Wq_w, dtype=np.float32)
    Wk_w = np.asarray(Wk_w, dtype=np.float32)
    Wv_w = np.asarray(Wv_w, dtype=np.float32)
    Wq_b = np.asarray(Wq_b, dtype=np.float32)
    Wk_b = np.asarray(Wk_b, dtype=np.float32)
    Wv_b = np.asarray(Wv_b, dtype=np.float32)

    in_maps = []
    for c in range(NCORES):
        b, g = divmod(c, 2)
        sl = slice(g * GEMB, (g + 1) * GEMB)
        in_maps.append({
            "xqt": np.ascontiguousarray(q_candidate[b].T.astype(np16)),
            "xkvt": np.ascontiguousarray(kv_candidate[b].T.astype(np16)),
            "wqt": np.ascontiguousarray(Wq_w[sl].T.astype(np16)),
            "wkt": np.ascontiguousarray(Wk_w[sl].T.astype(np16)),
            "wvt": np.ascontiguousarray(Wv_w[sl].T.astype(np16)),
            "qb": np.ascontiguousarray(Wq_b[sl]),
            "kb": np.ascontiguousarray(Wk_b[sl]),
            "vb": np.ascontiguousarray(Wv_b[sl].astype(np16)),
            "maskb": np.ascontiguousarray((pad_mask[b].astype(np.float32) * MASKVAL).astype(np16)),
        })
    return in_maps


def run(trace=False, **inputs):
    in_maps = _prep_in_maps(**inputs)
    nc = _build()
    res = run_bass_kernel_spmd(nc, in_maps, core_ids=list(range(NCORES)), trace=trace)
    out = np.empty((B, LQ, EMB), np.float32)
    att = np.empty((B, HEADS, LQ, LK), np.float32)
    for c in range(NCORES):
        b, g = divmod(c, 2)
        att[b, g * HPC:(g + 1) * HPC] = res.results[c]["att_part"]
        out[b, :, g * GEMB:(g + 1) * GEMB] = res.results[c]["out_part"]
    return (out, att), res


def kernel(**inputs):
    (out, att), _ = run(trace=False, **inputs)
    return out, att
